# revision 1
# baseline (speedup 1.0000x reference)
"""Approximate rank pooling (segment-reduce) on 8 TRN2 NeuronCores.

Strategy: the per-frame weight w[t] depends only on vidids (tiny), so it is
computed on the host exactly as the reference does. The heavy part -- the
weighted segment sum over x [2048, 3*128*128] -- runs on device as a matmul:
each core c gets an equal slice of 256 frames, x_c [256, 49152], plus a
host-built weight matrix W_c [256, NV] whose row i has w[global_t] at column
(vidids[global_t] - v_lo_c) and zeros elsewhere.  The core computes
out_c = W_c^T @ x_c with TensorEngine accumulating over the two 128-frame
K-chunks in PSUM.  The host then scatters/adds the per-core partial outputs
into the full [64, 3, 128, 128] result (a video straddling a core boundary
simply gets contributions from both cores).
"""

import numpy as np

T, C, H, W = 2048, 3, 128, 128
D = C * H * W              # 49152
NCORES = 8
TL = T // NCORES           # 256 frames per core
KP = 128                   # K chunk = SBUF partition count
NK = TL // KP              # 2 K-chunks
CHUNK = 4096               # columns of x per load (16 KiB contiguous rows --
                           # the per-SDMA-queue descriptor sweet spot)
NJ = D // CHUNK            # 12
SUB = 512                  # matmul moving-dim limit (fp32) = one PSUM bank
NSUB = CHUNK // SUB        # 8

MM_DTYPE = "float32r"      # "float32r" (1 cyc/row) or "float32" (4 cyc/row)


def _frame_weights(vid: np.ndarray, nvids: int) -> np.ndarray:
    """Replicates the reference weight math in numpy (float32)."""
    T_ = vid.shape[0]
    counts = np.bincount(vid, minlength=nvids).astype(np.int64)
    starts = np.cumsum(counts) - counts
    N = counts[vid]                                    # [T] segment size
    t = np.arange(T_, dtype=np.int64) - starts[vid] + 1  # [T] 1-based rank
    Hh = np.zeros(T_ + 1, dtype=np.float32)
    Hh[1:] = np.cumsum(
        (1.0 / np.arange(1, T_ + 1, dtype=np.float32)).astype(np.float32),
        dtype=np.float32,
    )
    poly = (N * (N + 1) - t * (t - 1) - N * (N - t + 1)).astype(np.float32)
    w = poly - (Hh[N] - Hh[t - 1])
    return np.where(N == 1, np.float32(1.0), w).astype(np.float32)


def _build_nc(nv: int, mm_dtype: str, last_store_eng: str = "gpsimd"):
    import concourse.bacc as bacc
    import concourse.tile as tile
    from concourse import mybir

    dt = getattr(mybir.dt, mm_dtype)
    f32 = mybir.dt.float32

    nc = bacc.Bacc("TRN2", target_bir_lowering=False, debug=False)
    x = nc.dram_tensor("x", [TL, D], dt, kind="ExternalInput").ap()
    wt = nc.dram_tensor("wt", [TL, nv], dt, kind="ExternalInput").ap()
    out = nc.dram_tensor("out", [nv, D], f32, kind="ExternalOutput").ap()

    with tile.TileContext(nc) as tc:
        with (
            tc.tile_pool(name="wpool", bufs=1) as wpool,
            tc.tile_pool(name="xpool", bufs=6) as xpool,
            tc.tile_pool(name="opool", bufs=2) as opool,
            tc.tile_pool(name="psum", bufs=8, space="PSUM") as ppool,
        ):
            wtiles = []
            for k in range(NK):
                wtile = wpool.tile([KP, nv], dt, tag=f"w{k}")
                nc.gpsimd.dma_start(wtile[:], wt[k * KP:(k + 1) * KP, :])
                wtiles.append(wtile)

            # All but the last CHUNK columns as full-width chunks; the final
            # chunk is split in half (and stored per PSUM bank) to shorten
            # the end-of-kernel load->matmul->copy->store chain.
            chunks = [(i * CHUNK, CHUNK) for i in range(NJ - 1)]
            chunks += [(D - CHUNK, CHUNK // 2), (D - CHUNK // 2, CHUNK // 2)]

            for ci, (col0, width) in enumerate(chunks):
                last = ci == len(chunks) - 1
                nsub = width // SUB
                xts = []
                for k in range(NK):
                    xt = xpool.tile([KP, CHUNK], dt, name="xt", tag="xt")
                    nc.sync.dma_start(
                        xt[:, :width],
                        x[k * KP:(k + 1) * KP, col0:col0 + width],
                    )
                    xts.append(xt)

                pts = [
                    ppool.tile([nv, SUB], f32, name="pt", tag="pt")
                    for _ in range(nsub)
                ]
                for k in range(NK):
                    for s in range(nsub):
                        nc.tensor.matmul(
                            pts[s][:],
                            wtiles[k][:],
                            xts[k][:, s * SUB:(s + 1) * SUB],
                            start=(k == 0),
                            stop=(k == NK - 1),
                        )

                ot = opool.tile([nv, CHUNK], f32, name="ot", tag="ot")
                for s in range(nsub):
                    nc.any.tensor_copy(ot[:, s * SUB:(s + 1) * SUB], pts[s][:])
                    if last:
                        getattr(nc, last_store_eng).dma_start(
                            out[:, col0 + s * SUB:col0 + (s + 1) * SUB],
                            ot[:, s * SUB:(s + 1) * SUB],
                        )
                if not last:
                    nc.gpsimd.dma_start(
                        out[:, col0:col0 + width], ot[:, :width]
                    )

    nc.compile()
    return nc


def _run(x, vidids, nvids, trace=False, trace_cores=None):
    from concourse.bass_utils import run_bass_kernel_spmd

    x = np.ascontiguousarray(np.asarray(x, dtype=np.float32))
    vid = np.asarray(vidids).astype(np.int64).ravel()
    nv_total = int(nvids)
    assert x.shape == (T, C, H, W) and vid.shape == (T,)

    w = _frame_weights(vid, nv_total)
    xf = x.reshape(T, D)

    v_lo, nv_local = [], []
    for c in range(NCORES):
        lo, hi = c * TL, (c + 1) * TL
        v_lo.append(int(vid[lo]))
        nv_local.append(int(vid[hi - 1]) - int(vid[lo]) + 1)
    NV = max(nv_local)

    in_maps = []
    rows = np.arange(TL)
    for c in range(NCORES):
        lo = c * TL
        Wc = np.zeros((TL, NV), dtype=np.float32)
        Wc[rows, vid[lo:lo + TL] - v_lo[c]] = w[lo:lo + TL]
        in_maps.append({"x": xf[lo:lo + TL], "wt": Wc})

    nc = _build_nc(NV, MM_DTYPE)
    res = run_bass_kernel_spmd(
        nc, in_maps, list(range(NCORES)), trace=trace, trace_cores=trace_cores
    )

    outf = np.zeros((nv_total, D), dtype=np.float32)
    for c in range(NCORES):
        part = res.results[c]["out"]
        n = min(NV, nv_total - v_lo[c])
        outf[v_lo[c]:v_lo[c] + n] += part[:n]
    return outf.reshape(nv_total, C, H, W), res


def kernel(x, vidids, nvids):
    out, _ = _run(x, vidids, nvids)
    return out



# revision 2
# speedup vs baseline: 2.0828x; 2.0828x over previous
"""Approximate rank pooling (segment-reduce) on 8 TRN2 NeuronCores.

Strategy: the per-frame weight w[t] depends only on vidids (tiny), so it is
computed on the host exactly as the reference does. The heavy part -- the
weighted segment sum over x [2048, 3*128*128] -- runs on device as a matmul:
each core c gets an equal slice of 256 frames, x_c [256, 49152], plus a
host-built weight matrix W_c [256, NV] whose row i has w[global_t] at column
(vidids[global_t] - v_lo_c) and zeros elsewhere.  The core computes
out_c = W_c^T @ x_c with TensorEngine accumulating over the two 128-frame
K-chunks in PSUM.  The host then scatters/adds the per-core partial outputs
into the full [64, 3, 128, 128] result (a video straddling a core boundary
simply gets contributions from both cores).

Precision: the rel-err budget is 2e-2.  x is fed as fp8 e3m4 (quantization
RMS ~1.34% on N(0,1) data -- measured 1.341% end-to-end on the actual
inputs), W as fp16 (exact to 2^-11), PSUM accumulates in fp32, and the
output is stored as fp16 (adds ~0.01%).  This cuts per-core HBM reads from
50.3 MB (f32) to 12.6 MB, moving the bottleneck to the TensorE streaming
floor (~41 us for 2x49152 moving columns at 1 col/cycle, 2.4 GHz warm).
"""

import numpy as np

T, C, H, W = 2048, 3, 128, 128
D = C * H * W              # 49152
NCORES = 8
TL = T // NCORES           # 256 frames per core
KP = 128                   # K chunk = SBUF partition count
NK = TL // KP              # 2 K-chunks
CHUNK = 4096               # columns of x per load (4 KiB fp8 rows/partition)
NJ = D // CHUNK            # 12
SUB = 512                  # matmul moving-dim limit for one fp32 PSUM bank
NSUB = CHUNK // SUB        # 8

X_DTYPE = "float8e3"       # e3m4: 1 B/elem, ~1.34% RMS quantization error
W_DTYPE = "float16"
OUT_DTYPE = "float16"


def _frame_weights(vid: np.ndarray, nvids: int) -> np.ndarray:
    """Replicates the reference weight math in numpy (float32)."""
    T_ = vid.shape[0]
    counts = np.bincount(vid, minlength=nvids).astype(np.int64)
    starts = np.cumsum(counts) - counts
    N = counts[vid]                                    # [T] segment size
    t = np.arange(T_, dtype=np.int64) - starts[vid] + 1  # [T] 1-based rank
    Hh = np.zeros(T_ + 1, dtype=np.float32)
    Hh[1:] = np.cumsum(
        (1.0 / np.arange(1, T_ + 1, dtype=np.float32)).astype(np.float32),
        dtype=np.float32,
    )
    poly = (N * (N + 1) - t * (t - 1) - N * (N - t + 1)).astype(np.float32)
    w = poly - (Hh[N] - Hh[t - 1])
    return np.where(N == 1, np.float32(1.0), w).astype(np.float32)


def _build_nc(nv: int, last_store_eng: str = "gpsimd"):
    import concourse.bacc as bacc
    import concourse.tile as tile
    from concourse import mybir

    xdt = getattr(mybir.dt, X_DTYPE)
    wdt = getattr(mybir.dt, W_DTYPE)
    odt = getattr(mybir.dt, OUT_DTYPE)

    nc = bacc.Bacc("TRN2", target_bir_lowering=False, debug=False)
    x = nc.dram_tensor("x", [TL, D], xdt, kind="ExternalInput").ap()
    wt = nc.dram_tensor("wt", [TL, nv], wdt, kind="ExternalInput").ap()
    out = nc.dram_tensor("out", [nv, D], odt, kind="ExternalOutput").ap()

    with tile.TileContext(nc) as tc:
        with (
            tc.tile_pool(name="wpool", bufs=1) as wpool,
            tc.tile_pool(name="xpool", bufs=6) as xpool,
            tc.tile_pool(name="opool", bufs=2) as opool,
            tc.tile_pool(name="psum", bufs=8, space="PSUM") as ppool,
        ):
            wtiles = []
            for k in range(NK):
                wtile = wpool.tile([KP, nv], wdt, tag=f"w{k}")
                nc.gpsimd.dma_start(wtile[:], wt[k * KP:(k + 1) * KP, :])
                wtiles.append(wtile)

            # All but the last CHUNK columns as full-width chunks; the final
            # chunk is split in half (and stored per PSUM bank) to shorten
            # the end-of-kernel load->matmul->copy->store chain.
            chunks = [(i * CHUNK, CHUNK) for i in range(NJ - 1)]
            chunks += [(D - CHUNK, CHUNK // 2), (D - CHUNK // 2, CHUNK // 2)]

            for ci, (col0, width) in enumerate(chunks):
                last = ci == len(chunks) - 1
                nsub = width // SUB
                xts = []
                for k in range(NK):
                    xt = xpool.tile([KP, CHUNK], xdt, name="xt", tag="xt")
                    nc.sync.dma_start(
                        xt[:, :width],
                        x[k * KP:(k + 1) * KP, col0:col0 + width],
                    )
                    xts.append(xt)

                pts = [
                    ppool.tile([nv, SUB], mybir.dt.float32, name="pt", tag="pt")
                    for _ in range(nsub)
                ]
                for k in range(NK):
                    for s in range(nsub):
                        nc.tensor.matmul(
                            pts[s][:],
                            wtiles[k][:],
                            xts[k][:, s * SUB:(s + 1) * SUB],
                            start=(k == 0),
                            stop=(k == NK - 1),
                        )

                ot = opool.tile([nv, CHUNK], odt, name="ot", tag="ot")
                for s in range(nsub):
                    nc.any.tensor_copy(ot[:, s * SUB:(s + 1) * SUB], pts[s][:])
                    if last:
                        getattr(nc, last_store_eng).dma_start(
                            out[:, col0 + s * SUB:col0 + (s + 1) * SUB],
                            ot[:, s * SUB:(s + 1) * SUB],
                        )
                if not last:
                    nc.gpsimd.dma_start(
                        out[:, col0:col0 + width], ot[:, :width]
                    )

    nc.compile()
    return nc


def _run(x, vidids, nvids, trace=False, trace_cores=None):
    import ml_dtypes
    from concourse.bass_utils import run_bass_kernel_spmd

    x = np.ascontiguousarray(np.asarray(x, dtype=np.float32))
    vid = np.asarray(vidids).astype(np.int64).ravel()
    nv_total = int(nvids)
    assert x.shape == (T, C, H, W) and vid.shape == (T,)

    w = _frame_weights(vid, nv_total)
    xq = x.reshape(T, D).astype(ml_dtypes.float8_e3m4)

    v_lo, nv_local = [], []
    for c in range(NCORES):
        lo, hi = c * TL, (c + 1) * TL
        v_lo.append(int(vid[lo]))
        nv_local.append(int(vid[hi - 1]) - int(vid[lo]) + 1)
    NV = max(nv_local)

    in_maps = []
    rows = np.arange(TL)
    for c in range(NCORES):
        lo = c * TL
        Wc = np.zeros((TL, NV), dtype=np.float32)
        Wc[rows, vid[lo:lo + TL] - v_lo[c]] = w[lo:lo + TL]
        in_maps.append({"x": xq[lo:lo + TL], "wt": Wc.astype(np.float16)})

    nc = _build_nc(NV)
    res = run_bass_kernel_spmd(
        nc, in_maps, list(range(NCORES)), trace=trace, trace_cores=trace_cores
    )

    outf = np.zeros((nv_total, D), dtype=np.float32)
    for c in range(NCORES):
        part = np.asarray(res.results[c]["out"]).astype(np.float32)
        n = min(NV, nv_total - v_lo[c])
        outf[v_lo[c]:v_lo[c] + n] += part[:n]
    return outf.reshape(nv_total, C, H, W), res


def kernel(x, vidids, nvids):
    out, _ = _run(x, vidids, nvids)
    return out


# revision 6
# speedup vs baseline: 2.5138x; 1.2069x over previous
"""Approximate rank pooling (segment-reduce) on 8 TRN2 NeuronCores.

Strategy: the per-frame weight w[t] depends only on vidids (tiny), so it is
computed on the host exactly as the reference does. The heavy part -- the
weighted segment sum over x [2048, 3*128*128] -- runs on device as a matmul:
each core c gets an equal slice of 256 frames, x_c [256, 49152], plus a
host-built weight matrix W_c [256, NV] whose row i has w[global_t] at column
(vidids[global_t] - v_lo_c) and zeros elsewhere.  The core computes
out_c = W_c^T @ x_c with TensorEngine accumulating over the two 128-frame
K-chunks in PSUM.  The host then scatters/adds the per-core partial outputs
into the full [64, 3, 128, 128] result (a video straddling a core boundary
simply gets contributions from both cores).

Precision: the rel-err budget is 2e-2.  x is fed as fp8 e3m4 (quantization
RMS ~1.34%, measured 1.341% end-to-end on the actual inputs), W as fp16,
PSUM accumulates in fp32, out is stored as fp16 (adds ~0.01%).  Per-core
HBM reads drop from 50.3 MB (f32) to 12.6 MB.

TensorE: with M=nv~12 only 12/128 PE columns are used, so NSTRIPS column
tiles (tile_position=(0,32j)) run concurrent matmuls on different column
slices of x, cutting the moving-stream floor from ~41us to ~41/NSTRIPS us.
The kernel is then DMA-bound (~13.8 MB at ~340 GB/s ~= 40 us/core).
"""

import numpy as np

T, C, H, W = 2048, 3, 128, 128
D = C * H * W              # 49152
NCORES = 8
TL = T // NCORES           # 256 frames per core
KP = 128                   # K chunk = SBUF partition count
NK = TL // KP              # 2 K-chunks
NSTRIPS = 3                # PE column tiles running concurrently
DS = D // NSTRIPS          # 16384 columns per strip
LOADW = 4096               # x-load width per strip per K-chunk (4 KiB lines)
NLOAD = DS // LOADW        # 4
PW = 2048                  # PSUM tile width (4 banks); 2 tiles = all 8 banks
NITER = DS // PW           # 8
SUB = 512                  # one fp32 PSUM bank

X_DTYPE = "float8e3"       # e3m4: 1 B/elem, ~1.34% RMS quantization error
W_DTYPE = "float16"
OUT_DTYPE = "float16"


def _frame_weights(vid: np.ndarray, nvids: int) -> np.ndarray:
    """Replicates the reference weight math in numpy (float32)."""
    T_ = vid.shape[0]
    counts = np.bincount(vid, minlength=nvids).astype(np.int64)
    starts = np.cumsum(counts) - counts
    N = counts[vid]                                    # [T] segment size
    t = np.arange(T_, dtype=np.int64) - starts[vid] + 1  # [T] 1-based rank
    Hh = np.zeros(T_ + 1, dtype=np.float32)
    Hh[1:] = np.cumsum(
        (1.0 / np.arange(1, T_ + 1, dtype=np.float32)).astype(np.float32),
        dtype=np.float32,
    )
    poly = (N * (N + 1) - t * (t - 1) - N * (N - t + 1)).astype(np.float32)
    w = poly - (Hh[N] - Hh[t - 1])
    return np.where(N == 1, np.float32(1.0), w).astype(np.float32)


def _build_nc(nv: int):
    import concourse.bacc as bacc
    import concourse.tile as tile
    from concourse import mybir

    assert nv <= 32, f"col-tiling needs nv<=32, got {nv}"
    xdt = getattr(mybir.dt, X_DTYPE)
    wdt = getattr(mybir.dt, W_DTYPE)
    odt = getattr(mybir.dt, OUT_DTYPE)
    f32 = mybir.dt.float32

    nc = bacc.Bacc("TRN2", target_bir_lowering=False, debug=False)
    x = nc.dram_tensor("x", [TL, D], xdt, kind="ExternalInput").ap()
    wt = nc.dram_tensor("wt", [TL, nv], wdt, kind="ExternalInput").ap()
    out = nc.dram_tensor("out", [nv, D], odt, kind="ExternalOutput").ap()

    cp_engines = ["vector", "scalar", "gpsimd"]

    with tile.TileContext(nc) as tc:
        with (
            tc.tile_pool(name="wpool", bufs=1) as wpool,
            tc.tile_pool(name="xpool", bufs=4 * NSTRIPS * NK) as xpool,
            tc.tile_pool(name="opool", bufs=2 * NSTRIPS) as opool,
            tc.tile_pool(name="psum", bufs=2, space="PSUM") as ppool,
        ):
            wtiles = []
            for k in range(NK):
                wtile = wpool.tile([KP, nv], wdt, tag=f"w{k}")
                nc.gpsimd.dma_start(wtile[:], wt[k * KP:(k + 1) * KP, :])
                wtiles.append(wtile)

            # x loads: [128, LOADW] fp8 per (load-iter, strip, k).
            xts = {}
            for li in range(NLOAD):
                for j in range(NSTRIPS):
                    for k in range(NK):
                        xt = xpool.tile([KP, LOADW], xdt, name="xt", tag="xt")
                        col0 = j * DS + li * LOADW
                        nc.sync.dma_start(
                            xt[:],
                            x[k * KP:(k + 1) * KP, col0:col0 + LOADW],
                        )
                        xts[(li, j, k)] = xt

            for ti in range(NITER):
                li, off = divmod(ti * PW, LOADW)
                pt = ppool.tile([KP, PW], f32, name="pt", tag="pt")
                for k in range(NK):
                    for s in range(PW // SUB):
                        for j in range(NSTRIPS):
                            nc.tensor.matmul(
                                pt[32 * j:32 * j + nv,
                                   s * SUB:(s + 1) * SUB],
                                wtiles[k][:],
                                xts[(li, j, k)][:, off + s * SUB:
                                                off + (s + 1) * SUB],
                                start=(k == 0),
                                stop=(k == NK - 1),
                                tile_position=(0, 32 * j),
                                skip_group_check=True,
                            )
                # Evacuate all strips at once: one [32*(NSTRIPS-1)+nv, PW]
                # copy covers every strip's partition range (per-partition
                # work is identical to copying a single strip), alternating
                # vector/scalar per iteration; the unwritten gap partitions
                # carry garbage that is simply never stored.
                np_copy = 32 * (NSTRIPS - 1) + nv
                ot = opool.tile([np_copy, PW], odt, name="ot", tag="ot")
                if ti % 2 == 0:
                    nc.vector.tensor_copy(ot[:], pt[:np_copy, :])
                else:
                    nc.scalar.copy(ot[:], pt[:np_copy, :])
                for j in range(NSTRIPS):
                    col0 = j * DS + ti * PW
                    nc.gpsimd.dma_start(
                        out[:, col0:col0 + PW],
                        ot[32 * j:32 * j + nv, :],
                    )

    nc.compile()
    return nc


def _run(x, vidids, nvids, trace=False, trace_cores=None):
    import ml_dtypes
    from concourse.bass_utils import run_bass_kernel_spmd

    x = np.ascontiguousarray(np.asarray(x, dtype=np.float32))
    vid = np.asarray(vidids).astype(np.int64).ravel()
    nv_total = int(nvids)
    assert x.shape == (T, C, H, W) and vid.shape == (T,)

    w = _frame_weights(vid, nv_total)
    xq = x.reshape(T, D).astype(ml_dtypes.float8_e3m4)

    v_lo, nv_local = [], []
    for c in range(NCORES):
        lo, hi = c * TL, (c + 1) * TL
        v_lo.append(int(vid[lo]))
        nv_local.append(int(vid[hi - 1]) - int(vid[lo]) + 1)
    NV = max(nv_local)

    in_maps = []
    rows = np.arange(TL)
    for c in range(NCORES):
        lo = c * TL
        Wc = np.zeros((TL, NV), dtype=np.float32)
        Wc[rows, vid[lo:lo + TL] - v_lo[c]] = w[lo:lo + TL]
        in_maps.append({"x": xq[lo:lo + TL], "wt": Wc.astype(np.float16)})

    nc = _build_nc(NV)
    res = run_bass_kernel_spmd(
        nc, in_maps, list(range(NCORES)), trace=trace, trace_cores=trace_cores
    )

    outf = np.zeros((nv_total, D), dtype=np.float32)
    for c in range(NCORES):
        part = np.asarray(res.results[c]["out"]).astype(np.float32)
        n = min(NV, nv_total - v_lo[c])
        outf[v_lo[c]:v_lo[c] + n] += part[:n]
    return outf.reshape(nv_total, C, H, W), res


def kernel(x, vidids, nvids):
    out, _ = _run(x, vidids, nvids)
    return out
